# revision 54
# baseline (speedup 1.0000x reference)
"""Trainium2 Bass kernel for the CPC contrastive loss problem.

Math (reference):
    fx = relu(x @ W1 + b1) @ W2 + b2          [N, Z]
    fz = z @ Wz + bz                          [N, Z]
    u[n] = fx[n] @ Ws[c[n]]                   [N, Z]
    T = softplus(<u, fz>_row)                 [N]
    neg_T[i] = mean_{j: c[j]==c[i]} softplus(<u[i], fz[j]>)
    out = log(T + eps) - log(neg_T + eps)

Key optimizations:
  - neg_T[i] only involves same-category j's (~N/C = 128 of 8192): rows are
    grouped by category on the host and S is computed in per-category 256-col
    blocks instead of the full NxN matrix.
  - softplus ~= relu inside the neg_T mean: |S| ~ N(0, ~100), so the
    log1p(exp(-|S|)) correction contributes ~1e-5 relative error to the
    output (verified numerically) while removing three full ACT passes
    (Abs/Exp/Ln) plus a TensorReduce per block.
  - x, z, W1, Wz travel as fp16 (halves the dominant DMA traffic; output
    rel err ~7e-4, well under the 2e-2 gate).  h1/u/fz/S stay fp32r.
  - Padded z columns hold zpad = -Wz^-T bz so that fz_pad = Wz^T zpad + bz
    ~= 0 exactly (fp16 rounding leaves ~1e-4): padded columns then add
    relu(~0) = 0 to the relu sums, with bz applied uniformly during the
    PSUM->SBUF copy.  No rank-1 bias matmul, no mask row.
  - S matmuls run in fp32r with 256-wide outputs (fp32r is full PE speed at
    ap_size >= 256), so no bf16 staging copies are needed.
  - One DMA per x block via a 3-dim access pattern (k-chunks in one
    transfer); HWDGE has a flat ~625ns serialized cost per DMA so the DMA
    count is kept to ~9.
  - The T-term log(T + 1e-8) needs ~1e-9 ABSOLUTE accuracy when d is very
    negative: computed on [128,16] via exp + branch-free log1p (series for
    e < 1e-4, Ln(1+e) otherwise).
"""

import sys

for _p in ("/opt/trn_rl_repo", "/root/.axon_site/_ro/trn_rl_repo"):
    if _p not in sys.path:
        sys.path.append(_p)

import numpy as np

import concourse.bacc as bacc
import concourse.tile as tile
from concourse import mybir as mb
from concourse.bass_utils import run_bass_kernel_spmd

# ---------------------------------------------------------------- constants
N, IN, Z, C, H = 8192, 512, 128, 64, 50
NCORES = 8
G = C // NCORES          # categories per core
B = 256                  # bucket (padded category) size
R = G * B                # padded rows per core = 2048
NCHUNK = R // 128        # 16 row-chunks of 128
KX = IN // 128           # 4 k-tiles for x
EPS = 1e-8

# packed small-weight layout (fp32 columns of the [128, PW] packB tensor).
# Weights are fp16 (the standalone-Ldweights split keys on the IFMAP dtype,
# not the weights dtype, so fp16 lhsT costs nothing extra on PE SEQ).
PK_W1 = (0, 100)         # fp16x2: [(k p) h -> p (k h)] = 200 fp16 cols
PK_WZ = (100, 164)       # fp16x2: Wz [128, 128]
PK_BS = (164, 172)       # b2 @ Ws[g], one column per g (fp32)
PK_B1 = (172, 173)       # rows 0:50 (fp32)
PK_BZ = (173, 174)       # bz column (fp32)
PW = 174

N_WARM = 4               # PE warm-up matmul count
JW = 176                 # relu row-sum width (max category size; host
                         # falls back to numpy above this)
XBLOCKS = [(0, 1), (1, 1), (2, 2), (4, 2), (6, 2)]  # (first cat, ncat)

F = mb.ActivationFunctionType
OP = mb.AluOpType
FP32 = mb.dt.float32
FP32R = mb.dt.float32r
FP16 = mb.dt.float16

_PROGRAM = None


def _build_program():
    nc = bacc.Bacc("TRN2", target_bir_lowering=False, debug=False)

    d_xgT = nc.dram_tensor("xgT", [IN, R], FP16, kind="ExternalInput").ap()
    d_zgT = nc.dram_tensor("zgT", [Z, R], FP16, kind="ExternalInput").ap()
    d_packB = nc.dram_tensor("packB", [128, PW], FP32, kind="ExternalInput").ap()
    d_w2s = nc.dram_tensor("w2s", [H, G * Z], FP16, kind="ExternalInput").ap()
    d_dr = nc.dram_tensor("dr", [128, 2 * NCHUNK], FP32, kind="ExternalOutput").ap()

    with tile.TileContext(nc) as tc:
        with (
            tc.tile_pool(name="const", bufs=1) as const,
            tc.tile_pool(name="junk", bufs=2) as junkp,
            tc.tile_pool(name="psum_mlp", bufs=1, space="PSUM") as psum_mlp,
            tc.tile_pool(name="psum_s", bufs=1, space="PSUM") as psum_sp,
        ):
            # ---- constants
            s_ones = const.tile([128, 1], FP32)
            nc.vector.memset(s_ones[:], 1.0)
            # Pre-load the ONE ACT table set containing every function this
            # kernel uses (Abs/Exp/Ln/Relu all live in
            # natural_log_exp_and_others, act_func_set_id 6).
            nc.scalar.add_instruction(
                mb.InstLoadActFuncSet(
                    name=nc.get_next_instruction_name(),
                    ins=[],
                    outs=[],
                    act_func_set_id=6,
                )
            )
            s_warmact = const.tile([128, 1], FP32)
            nc.scalar.activation(out=s_warmact[:], in_=s_ones[:], func=F.Abs)

            # ---- DMAs. HWDGE costs ~625ns serialized per DMA and the DMA
            # engines transfer strictly one DMA at a time, so both the COUNT
            # and the ORDER matter: everything is sequenced by first use,
            # with z slices landing just ahead of their x blocks.
            s_packB = const.tile([128, PW], FP32)
            nc.sync.dma_start(out=s_packB[:], in_=d_packB[:])

            s_zgT = const.tile([128, R], FP16)
            s_xgT = const.tile([128, KX, R], FP16)
            s_w2s = const.tile([H, G * Z], FP16)
            x_view = d_xgT.rearrange("(k p) n -> p k n", p=128)

            def dma_x(bi):
                g0, ncat = XBLOCKS[bi]
                ns = slice(g0 * B, (g0 + ncat) * B)
                nc.sync.dma_start(out=s_xgT[:, :, ns], in_=x_view[:, :, ns])

            def dma_z(c0, c1):
                zs = slice(c0 * B, c1 * B)
                nc.sync.dma_start(out=s_zgT[:, zs], in_=d_zgT[:, zs])

            dma_z(0, 2)
            dma_x(0)
            dma_x(1)
            nc.sync.dma_start(out=s_w2s[:], in_=d_w2s[:])
            dma_z(2, 4)
            dma_x(2)
            dma_z(4, 6)
            dma_x(3)
            dma_z(6, 8)
            dma_x(4)

            # packed views
            s_w1 = (
                s_packB[:, PK_W1[0] : PK_W1[1]]
                .bitcast(FP16)
                .rearrange("p (k h) -> p k h", k=KX)
            )
            s_wz = s_packB[:, PK_WZ[0] : PK_WZ[1]].bitcast(FP16)
            s_bs = s_packB[:, PK_BS[0] : PK_BS[1]]
            s_b1 = s_packB[0:H, PK_B1[0] : PK_B1[1]]
            s_bz = s_packB[:, PK_BZ[0] : PK_BZ[1]]

            # [128,128] identity mask for extracting diag(S) = d per chunk
            s_onesB = const.tile([128, 128], FP32)
            nc.vector.memset(s_onesB[:], 1.0)
            s_diag = const.tile([128, 128], FP32)
            nc.gpsimd.affine_select(
                out=s_diag[:], in_=s_onesB[:], pattern=[[1, 128]],
                compare_op=OP.is_equal, fill=0.0, base=0,
                channel_multiplier=-1,
            )



            # ---- persistent tiles.  s_dr packs [diag(S) | relu-sums]
            # so a single DMA ships both to the host, which finishes the
            # tiny scalar epilogue (softplus/log) in float64.
            s_h1T = const.tile([H, R], FP32R)
            s_fzT = const.tile([128, R], FP32R)
            s_uT = const.tile([128, R], FP32R)
            s_dr = const.tile([128, 2 * NCHUNK], FP32)

            # explicit slot-sliced PSUM tiles (8 banks exactly):
            #   pz1 [128,1,512] 1 bank | ph2 [50,2,512] 2 | pu4 [128,4,256] 2
            #   pS6 [128,6,256] 3
            pz1 = psum_mlp.tile([128, 512], FP32, tag="pz")
            ph2 = psum_mlp.tile([H, 2, 512], FP32, tag="ph")
            pu4 = psum_mlp.tile([128, 4, B], FP32, tag="pu")
            pS6 = psum_sp.tile([128, 6, B], FP32, tag="ps")

            # PE warm-up: tiny matmuls so the p-state ramp clock starts now
            # (targets a pS6 corner; real S matmuls overwrite it with start=1)
            s_wrhs = const.tile([128, 8], FP32)
            nc.vector.memset(s_wrhs[:], 0.0)
            for _ in range(N_WARM):
                nc.tensor.matmul(
                    pS6[0:1, 0, 0:8], lhsT=s_ones[:], rhs=s_wrhs[:],
                    start=True, stop=True,
                )

            # ---- main loop over category blocks.
            # Engine roles: PE matmuls | ACT h1-relu + fz-bias | DVE and
            # Pool split u-bias, relu row-sums and diag extractions.
            # The terminal reductions (row-sums/diags) of block b are issued
            # only after block b+1's mid-chain ops so that, with strictly
            # in-order engine queues, a terminal op never sits ahead of the
            # next block's u-bias on DVE/Pool.  pS has 6 slots so an S
            # matmul's PSUM-slot reuse never waits on a deferred reduction.
            def issue_fz(bi):
                g0, ncat = XBLOCKS[bi]
                w = ncat * B
                ns = slice(g0 * B, g0 * B + w)
                pz = pz1[:, 0:w]
                nc.tensor.matmul(
                    pz, lhsT=s_wz, rhs=s_zgT[:, ns], start=True, stop=True
                )
                # +bz on copy-out; padded z cols hold zpad so padded fz ~= 0
                nc.scalar.activation(
                    out=s_fzT[:, ns], in_=pz, func=F.Identity, bias=s_bz
                )

            def issue_reductions(chunks, engs=None):
                for i, ci in enumerate(chunks):
                    h = ci % 2
                    # relu row-sum straight from PSUM (only cols 0:JW are
                    # real, the rest are zero-padded)
                    jk = junkp.tile([128, JW], FP32, tag="junk")
                    eng = (engs or (nc.vector, nc.gpsimd))[i % 2]
                    eng.tensor_scalar(
                        out=jk[:],
                        in0=pS6[:, ci % 6, 0:JW],
                        scalar1=0.0,
                        scalar2=None,
                        op0=OP.max,
                        op1=OP.add,
                        accum_out=s_dr[:, NCHUNK + ci : NCHUNK + ci + 1],
                    )
                    # d = diag(S): the T-term diagonal sits in cols
                    # [h*128, h*128+128) of the chunk; mask + row-sum
                    jd = junkp.tile([128, 128], FP32, tag="junkd")
                    engd = (engs or (nc.gpsimd, nc.vector))[(i + 1) % 2]
                    engd.scalar_tensor_tensor(
                        out=jd[:],
                        in0=pS6[:, ci % 6, h * 128 : h * 128 + 128],
                        scalar=1.0,
                        in1=s_diag[:],
                        op0=OP.mult,
                        op1=OP.mult,
                        accum_out=s_dr[:, ci : ci + 1],
                    )

            issue_fz(0)
            pending = []
            for bi, (g0, ncat) in enumerate(XBLOCKS):
                w = ncat * B
                ns = slice(g0 * B, g0 * B + w)

                # h1 = relu(W1^T x + b1) on ACT (per-partition bias + relu)
                ph = ph2[:, bi % 2, 0:w]
                for k in range(KX):
                    nc.tensor.matmul(
                        ph,
                        lhsT=s_w1[:, k, :],
                        rhs=s_xgT[:, k, ns],
                        start=(k == 0),
                        stop=(k == KX - 1),
                    )
                nc.scalar.activation(
                    out=s_h1T[:, ns], in_=ph, func=F.Relu, bias=s_b1
                )

                # u matmuls, then the u-bias copies off the same pu slot
                for gg in range(ncat):
                    g = g0 + gg
                    gs = slice(g * B, (g + 1) * B)
                    nc.tensor.matmul(
                        pu4[:, g % 4, :],
                        lhsT=s_w2s[:, g * Z : (g + 1) * Z],
                        rhs=s_h1T[:, gs],
                        start=True,
                        stop=True,
                    )
                for gg in range(ncat):
                    g = g0 + gg
                    gs = slice(g * B, (g + 1) * B)
                    engu = nc.vector if g % 2 == 0 else nc.gpsimd
                    engu.tensor_scalar_add(
                        s_uT[:, gs], pu4[:, g % 4, :], s_bs[:, g : g + 1]
                    )
                if bi + 1 < len(XBLOCKS):
                    issue_fz(bi + 1)

                for gg in range(ncat):
                    g = g0 + gg
                    gs = slice(g * B, (g + 1) * B)
                    for h in range(2):
                        ci = 2 * g + h
                        # S chunk, fp32r (full PE speed at 256-wide output)
                        nc.tensor.matmul(
                            pS6[:, ci % 6, :],
                            lhsT=s_uT[:, g * B + h * 128 : g * B + (h + 1) * 128],
                            rhs=s_fzT[:, gs],
                            start=True,
                            stop=True,
                        )
                # previous block's reductions run now, strictly behind this
                # block's u-bias ops in the DVE/Pool queues
                issue_reductions(pending)
                pending = [2 * (g0 + gg) + h for gg in range(ncat) for h in range(2)]

            # last block's reductions drain with ACT helping on the row-sums
            issue_reductions(pending)
            nc.sync.dma_start(out=d_dr[:], in_=s_dr[:])

    nc.compile()
    return nc


def get_program():
    global _PROGRAM
    if _PROGRAM is None:
        _PROGRAM = _build_program()
    return _PROGRAM


# ---------------------------------------------------------------- host side
def _pack_weights(W1, b1, Wz, bz, W2, b2, Ws):
    """Core-independent packed weights: packB minus pinv, plus per-core w2s."""
    packB = np.zeros((128, PW), np.float32)
    w1h = (
        W1.reshape(KX, 128, H).transpose(1, 0, 2).reshape(128, KX * H)
    ).astype(np.float16)
    packB[:, PK_W1[0] : PK_W1[1]] = w1h.view(np.float32)
    packB[:, PK_WZ[0] : PK_WZ[1]] = Wz.astype(np.float16).view(np.float32)
    packB[:H, PK_B1[0]] = b1
    packB[:, PK_BZ[0]] = bz
    return packB


def _prep_core_inputs(x16, z16, zpad16, packB_base, w2s_all, bs_all, idx_lists, core):
    """Per-core input map (grouped, padded, transposed, packed)."""
    xgT = np.zeros((IN, R), np.float16)
    zgT = np.empty((Z, R), np.float16)
    zgT[:] = zpad16[:, None]
    for s in range(G):
        k = core * G + s
        idx = idx_lists[k]
        n = len(idx)
        lo = s * B
        if n:
            xgT[:, lo : lo + n] = x16[idx].T
            zgT[:, lo : lo + n] = z16[idx].T
    packB = packB_base.copy()
    packB[:, PK_BS[0] : PK_BS[1]] = bs_all[core * G : (core + 1) * G].T
    w2s = w2s_all[core]
    return {"xgT": xgT, "zgT": zgT, "packB": packB, "w2s": w2s}


def _numpy_fallback(x, c, z, W1, b1, W2, b2, Wz, bz, Ws):
    x64 = x.astype(np.float64)
    fx = np.maximum(x64 @ W1.astype(np.float64) + b1, 0.0) @ W2.astype(
        np.float64
    ) + b2
    fz = z.astype(np.float64) @ Wz.astype(np.float64) + bz
    u = np.einsum("nd,nde->ne", fx, Ws.astype(np.float64)[c])

    def sp(v):
        return np.log1p(np.exp(-np.abs(v))) + np.maximum(v, 0.0)

    T = sp(np.einsum("ne,ne->n", u, fz))
    out = np.empty(N, np.float64)
    for k in range(C):
        idx = np.where(c == k)[0]
        if len(idx) == 0:
            continue
        Sk = sp(u[idx] @ fz[idx].T)
        neg = Sk.mean(axis=1)
        out[idx] = np.log(T[idx] + EPS) - np.log(neg + EPS)
    return out.astype(np.float32)


def _host_prepare(x, cf, z, W1, b1, W2, b2, Wz, bz, Ws, idx_lists):
    """Build per-core input maps; returns None if the fallback must run."""
    try:
        zpad = -np.linalg.solve(Wz.astype(np.float64).T, bz.astype(np.float64))
    except np.linalg.LinAlgError:
        return None
    if not np.all(np.isfinite(zpad)) or np.abs(zpad).max() > 1e3:
        return None
    zpad16 = zpad.astype(np.float16)

    packB_base = _pack_weights(W1, b1, Wz, bz, W2, b2, Ws)
    # fold the second MLP layer into each category's bilinear weight:
    # u = relu(h1) @ (W2 Ws[g]) + b2 Ws[g]
    Ws64 = Ws.astype(np.float64)
    w2s_full = np.einsum("he,cef->chf", W2.astype(np.float64), Ws64)
    bs_all = (b2.astype(np.float64) @ Ws64).astype(np.float32)  # [C, Z]
    w2s_all = [
        np.ascontiguousarray(
            w2s_full[core * G : (core + 1) * G]
            .transpose(1, 0, 2)
            .reshape(H, G * Z),
            dtype=np.float16,
        )
        for core in range(NCORES)
    ]
    x16 = x.astype(np.float16)
    z16 = z.astype(np.float16)

    return [
        _prep_core_inputs(
            x16, z16, zpad16, packB_base, w2s_all, bs_all, idx_lists, core
        )
        for core in range(NCORES)
    ]


def kernel(x, c, z, W1, b1, W2, b2, Wz, bz, Ws):
    x = np.ascontiguousarray(np.asarray(x), dtype=np.float32)
    z = np.ascontiguousarray(np.asarray(z), dtype=np.float32)
    W1 = np.ascontiguousarray(np.asarray(W1), dtype=np.float32)
    b1 = np.ascontiguousarray(np.asarray(b1), dtype=np.float32)
    W2 = np.ascontiguousarray(np.asarray(W2), dtype=np.float32)
    b2 = np.ascontiguousarray(np.asarray(b2), dtype=np.float32)
    Wz = np.ascontiguousarray(np.asarray(Wz), dtype=np.float32)
    bz = np.ascontiguousarray(np.asarray(bz), dtype=np.float32)
    Ws = np.ascontiguousarray(np.asarray(Ws), dtype=np.float32)
    cf = np.asarray(c).reshape(-1).astype(np.int64)

    idx_lists = [np.where(cf == k)[0] for k in range(C)]
    if max(len(i) for i in idx_lists) > JW:
        return _numpy_fallback(x, cf, z, W1, b1, W2, b2, Wz, bz, Ws)

    in_maps = _host_prepare(x, cf, z, W1, b1, W2, b2, Wz, bz, Ws, idx_lists)
    if in_maps is None:
        return _numpy_fallback(x, cf, z, W1, b1, W2, b2, Wz, bz, Ws)

    nc = get_program()
    res = run_bass_kernel_spmd(nc, in_maps, core_ids=list(range(NCORES)))

    # scalar epilogue in float64 on the host: y = log(softplus(d) + eps)
    #                                            - log(relu_sum / n + eps)
    out = np.empty(N, np.float32)
    for core in range(NCORES):
        dr = res.results[core]["dr"].astype(np.float64)  # [128, 2*NCHUNK]
        d = np.ascontiguousarray(dr[:, :NCHUNK].T).reshape(R)
        rel = np.ascontiguousarray(dr[:, NCHUNK:].T).reshape(R)
        T = np.log1p(np.exp(-np.abs(d))) + np.maximum(d, 0.0)
        for s in range(G):
            k = core * G + s
            idx = idx_lists[k]
            n = len(idx)
            if n:
                sl = slice(s * B, s * B + n)
                y = np.log(T[sl] + EPS) - np.log(rel[sl] / n + EPS)
                out[idx] = y.astype(np.float32)
    return out


# revision 55
# speedup vs baseline: 1.0389x; 1.0389x over previous
"""Trainium2 Bass kernel for the CPC contrastive loss problem.

Math (reference):
    fx = relu(x @ W1 + b1) @ W2 + b2          [N, Z]
    fz = z @ Wz + bz                          [N, Z]
    u[n] = fx[n] @ Ws[c[n]]                   [N, Z]
    T = softplus(<u, fz>_row)                 [N]
    neg_T[i] = mean_{j: c[j]==c[i]} softplus(<u[i], fz[j]>)
    out = log(T + eps) - log(neg_T + eps)

Key optimizations:
  - neg_T[i] only involves same-category j's (~N/C = 128 of 8192): rows are
    grouped by category on the host and S is computed in per-category 256-col
    blocks instead of the full NxN matrix.
  - softplus ~= relu inside the neg_T mean: |S| ~ N(0, ~100), so the
    log1p(exp(-|S|)) correction contributes ~1e-5 relative error to the
    output (verified numerically) while removing three full ACT passes
    (Abs/Exp/Ln) plus a TensorReduce per block.
  - x, z, W1, Wz travel as fp16 (halves the dominant DMA traffic; output
    rel err ~7e-4, well under the 2e-2 gate).  h1/u/fz/S stay fp32r.
  - Padded z columns hold zpad = -Wz^-T bz so that fz_pad = Wz^T zpad + bz
    ~= 0 exactly (fp16 rounding leaves ~1e-4): padded columns then add
    relu(~0) = 0 to the relu sums, with bz applied uniformly during the
    PSUM->SBUF copy.  No rank-1 bias matmul, no mask row.
  - S matmuls run in fp32r with 256-wide outputs (fp32r is full PE speed at
    ap_size >= 256), so no bf16 staging copies are needed.
  - One DMA per x block via a 3-dim access pattern (k-chunks in one
    transfer); HWDGE has a flat ~625ns serialized cost per DMA so the DMA
    count is kept to ~9.
  - The T-term log(T + 1e-8) needs ~1e-9 ABSOLUTE accuracy when d is very
    negative: computed on [128,16] via exp + branch-free log1p (series for
    e < 1e-4, Ln(1+e) otherwise).
"""

import sys

for _p in ("/opt/trn_rl_repo", "/root/.axon_site/_ro/trn_rl_repo"):
    if _p not in sys.path:
        sys.path.append(_p)

import numpy as np

import concourse.bacc as bacc
import concourse.tile as tile
from concourse import mybir as mb
from concourse.bass_utils import run_bass_kernel_spmd

# ---------------------------------------------------------------- constants
N, IN, Z, C, H = 8192, 512, 128, 64, 50
NCORES = 8
G = C // NCORES          # categories per core
B = 256                  # bucket (padded category) size
R = G * B                # padded rows per core = 2048
NCHUNK = R // 128        # 16 row-chunks of 128
KX = IN // 128           # 4 k-tiles for x
EPS = 1e-8

# packed small-weight layout (fp32 columns of the [128, PW] packB tensor).
# Weights are fp16 (the standalone-Ldweights split keys on the IFMAP dtype,
# not the weights dtype, so fp16 lhsT costs nothing extra on PE SEQ).
PK_W1 = (0, 100)         # fp16x2: [(k p) h -> p (k h)] = 200 fp16 cols
PK_WZ = (100, 164)       # fp16x2: Wz [128, 128]
PK_BS = (164, 172)       # b2 @ Ws[g], one column per g (fp32)
PK_B1 = (172, 173)       # rows 0:50 (fp32)
PK_BZ = (173, 174)       # bz column (fp32)
PW = 174

N_WARM = 4               # PE warm-up matmul count
JW = 176                 # relu row-sum width (max category size; host
                         # falls back to numpy above this)
XBLOCKS = [(0, 1), (1, 1), (2, 2), (4, 2), (6, 2)]  # (first cat, ncat)

F = mb.ActivationFunctionType
OP = mb.AluOpType
FP32 = mb.dt.float32
FP32R = mb.dt.float32r
FP16 = mb.dt.float16

_PROGRAM = None


def _build_program():
    nc = bacc.Bacc("TRN2", target_bir_lowering=False, debug=False)

    d_xgT = nc.dram_tensor("xgT", [IN, R], FP16, kind="ExternalInput").ap()
    d_zgT = nc.dram_tensor("zgT", [Z, R], FP16, kind="ExternalInput").ap()
    d_packB = nc.dram_tensor("packB", [128, PW], FP32, kind="ExternalInput").ap()
    d_w2s = nc.dram_tensor("w2s", [H, G * Z], FP16, kind="ExternalInput").ap()
    d_dr = nc.dram_tensor("dr", [128, 2 * NCHUNK], FP32, kind="ExternalOutput").ap()

    with tile.TileContext(nc) as tc:
        with (
            tc.tile_pool(name="const", bufs=1) as const,
            tc.tile_pool(name="junk", bufs=2) as junkp,
            tc.tile_pool(name="psum_mlp", bufs=1, space="PSUM") as psum_mlp,
            tc.tile_pool(name="psum_s", bufs=1, space="PSUM") as psum_sp,
        ):
            # ---- constants
            s_ones = const.tile([128, 1], FP32)
            nc.vector.memset(s_ones[:], 1.0)
            # Pre-load the ONE ACT table set containing every function this
            # kernel uses (Abs/Exp/Ln/Relu all live in
            # natural_log_exp_and_others, act_func_set_id 6).
            nc.scalar.add_instruction(
                mb.InstLoadActFuncSet(
                    name=nc.get_next_instruction_name(),
                    ins=[],
                    outs=[],
                    act_func_set_id=6,
                )
            )
            s_warmact = const.tile([128, 1], FP32)
            nc.scalar.activation(out=s_warmact[:], in_=s_ones[:], func=F.Abs)

            # ---- DMAs. HWDGE costs ~625ns serialized per DMA and the DMA
            # engines transfer strictly one DMA at a time, so both the COUNT
            # and the ORDER matter: everything is sequenced by first use,
            # with z slices landing just ahead of their x blocks.
            s_packB = const.tile([128, PW], FP32)
            nc.sync.dma_start(out=s_packB[:], in_=d_packB[:])

            s_zgT = const.tile([128, R], FP16)
            s_xgT = const.tile([128, KX, R], FP16)
            s_w2s = const.tile([H, G * Z], FP16)
            x_view = d_xgT.rearrange("(k p) n -> p k n", p=128)

            def dma_x(bi):
                g0, ncat = XBLOCKS[bi]
                ns = slice(g0 * B, (g0 + ncat) * B)
                nc.sync.dma_start(out=s_xgT[:, :, ns], in_=x_view[:, :, ns])

            def dma_z(c0, c1):
                zs = slice(c0 * B, c1 * B)
                nc.sync.dma_start(out=s_zgT[:, zs], in_=d_zgT[:, zs])

            dma_z(0, 2)
            dma_x(0)
            dma_x(1)
            nc.sync.dma_start(out=s_w2s[:], in_=d_w2s[:])
            dma_z(2, 4)
            dma_x(2)
            dma_z(4, 6)
            dma_x(3)
            dma_z(6, 8)
            dma_x(4)

            # packed views
            s_w1 = (
                s_packB[:, PK_W1[0] : PK_W1[1]]
                .bitcast(FP16)
                .rearrange("p (k h) -> p k h", k=KX)
            )
            s_wz = s_packB[:, PK_WZ[0] : PK_WZ[1]].bitcast(FP16)
            s_bs = s_packB[:, PK_BS[0] : PK_BS[1]]
            s_b1 = s_packB[0:H, PK_B1[0] : PK_B1[1]]
            s_bz = s_packB[:, PK_BZ[0] : PK_BZ[1]]

            # [128,128] identity mask for extracting diag(S) = d per chunk
            s_onesB = const.tile([128, 128], FP32)
            nc.vector.memset(s_onesB[:], 1.0)
            s_diag = const.tile([128, 128], FP32)
            nc.gpsimd.affine_select(
                out=s_diag[:], in_=s_onesB[:], pattern=[[1, 128]],
                compare_op=OP.is_equal, fill=0.0, base=0,
                channel_multiplier=-1,
            )



            # ---- persistent tiles.  s_dr packs [diag(S) | relu-sums]
            # so a single DMA ships both to the host, which finishes the
            # tiny scalar epilogue (softplus/log) in float64.
            s_h1T = const.tile([H, R], FP32R)
            s_fzT = const.tile([128, R], FP32R)
            s_uT = const.tile([128, R], FP32R)
            s_dr = const.tile([128, 2 * NCHUNK], FP32)

            # explicit slot-sliced PSUM tiles (8 banks exactly):
            #   pz1 [128,1,512] 1 bank | ph2 [50,2,512] 2 | pu4 [128,4,256] 2
            #   pS6 [128,6,256] 3
            pz1 = psum_mlp.tile([128, 512], FP32, tag="pz")
            ph2 = psum_mlp.tile([H, 2, 512], FP32, tag="ph")
            pu4 = psum_mlp.tile([128, 4, B], FP32, tag="pu")
            pS6 = psum_sp.tile([128, 6, B], FP32, tag="ps")

            # PE warm-up: tiny matmuls so the p-state ramp clock starts now
            # (targets a pS6 corner; real S matmuls overwrite it with start=1)
            s_wrhs = const.tile([128, 8], FP32)
            nc.vector.memset(s_wrhs[:], 0.0)
            for _ in range(N_WARM):
                nc.tensor.matmul(
                    pS6[0:1, 0, 0:8], lhsT=s_ones[:], rhs=s_wrhs[:],
                    start=True, stop=True,
                )

            # ---- main loop over category blocks.
            # Engine roles: PE matmuls | ACT h1-relu + fz-bias | DVE and
            # Pool split u-bias, relu row-sums and diag extractions.
            # The terminal reductions (row-sums/diags) of block b are issued
            # only after block b+1's mid-chain ops so that, with strictly
            # in-order engine queues, a terminal op never sits ahead of the
            # next block's u-bias on DVE/Pool.  pS has 6 slots so an S
            # matmul's PSUM-slot reuse never waits on a deferred reduction.
            def issue_fz(bi):
                g0, ncat = XBLOCKS[bi]
                w = ncat * B
                ns = slice(g0 * B, g0 * B + w)
                pz = pz1[:, 0:w]
                nc.tensor.matmul(
                    pz, lhsT=s_wz, rhs=s_zgT[:, ns], start=True, stop=True
                )
                # +bz on copy-out; padded z cols hold zpad so padded fz ~= 0
                nc.scalar.activation(
                    out=s_fzT[:, ns], in_=pz, func=F.Identity, bias=s_bz
                )

            def issue_reductions(chunks, engs=None):
                for i, ci in enumerate(chunks):
                    h = ci % 2
                    # relu row-sum straight from PSUM (only cols 0:JW are
                    # real, the rest are zero-padded)
                    jk = junkp.tile([128, JW], FP32, tag="junk")
                    eng = (engs or (nc.vector, nc.gpsimd))[i % 2]
                    eng.tensor_scalar(
                        out=jk[:],
                        in0=pS6[:, ci % 6, 0:JW],
                        scalar1=0.0,
                        scalar2=None,
                        op0=OP.max,
                        op1=OP.add,
                        accum_out=s_dr[:, NCHUNK + ci : NCHUNK + ci + 1],
                    )
                    # d = diag(S): the T-term diagonal sits in cols
                    # [h*128, h*128+128) of the chunk; mask + row-sum
                    jd = junkp.tile([128, 128], FP32, tag="junkd")
                    engd = (engs or (nc.gpsimd, nc.vector))[(i + 1) % 2]
                    engd.scalar_tensor_tensor(
                        out=jd[:],
                        in0=pS6[:, ci % 6, h * 128 : h * 128 + 128],
                        scalar=1.0,
                        in1=s_diag[:],
                        op0=OP.mult,
                        op1=OP.mult,
                        accum_out=s_dr[:, ci : ci + 1],
                    )

            issue_fz(0)
            pending = []
            for bi, (g0, ncat) in enumerate(XBLOCKS):
                w = ncat * B
                ns = slice(g0 * B, g0 * B + w)

                # h1 = relu(W1^T x + b1) on ACT (per-partition bias + relu)
                ph = ph2[:, bi % 2, 0:w]
                for k in range(KX):
                    nc.tensor.matmul(
                        ph,
                        lhsT=s_w1[:, k, :],
                        rhs=s_xgT[:, k, ns],
                        start=(k == 0),
                        stop=(k == KX - 1),
                    )
                nc.scalar.activation(
                    out=s_h1T[:, ns], in_=ph, func=F.Relu, bias=s_b1
                )

                # u matmuls, then the u-bias copies off the same pu slot
                for gg in range(ncat):
                    g = g0 + gg
                    gs = slice(g * B, (g + 1) * B)
                    nc.tensor.matmul(
                        pu4[:, g % 4, :],
                        lhsT=s_w2s[:, g * Z : (g + 1) * Z],
                        rhs=s_h1T[:, gs],
                        start=True,
                        stop=True,
                    )
                for gg in range(ncat):
                    g = g0 + gg
                    gs = slice(g * B, (g + 1) * B)
                    engu = nc.vector if g % 2 == 0 else nc.gpsimd
                    engu.tensor_scalar_add(
                        s_uT[:, gs], pu4[:, g % 4, :], s_bs[:, g : g + 1]
                    )
                if bi + 1 < len(XBLOCKS):
                    issue_fz(bi + 1)

                # previous block's reductions: issued behind this block's
                # u-bias ops (queue order) but ahead of its S matmuls (so
                # each pS slot is still holding the chunk they must read)
                issue_reductions(pending)
                pending = [2 * (g0 + gg) + h for gg in range(ncat) for h in range(2)]

                for gg in range(ncat):
                    g = g0 + gg
                    gs = slice(g * B, (g + 1) * B)
                    for h in range(2):
                        ci = 2 * g + h
                        # S chunk, fp32r (full PE speed at 256-wide output)
                        nc.tensor.matmul(
                            pS6[:, ci % 6, :],
                            lhsT=s_uT[:, g * B + h * 128 : g * B + (h + 1) * 128],
                            rhs=s_fzT[:, gs],
                            start=True,
                            stop=True,
                        )

            # last block's reductions drain with ACT helping on the row-sums
            issue_reductions(pending)
            nc.sync.dma_start(out=d_dr[:], in_=s_dr[:])

    nc.compile()
    return nc


def get_program():
    global _PROGRAM
    if _PROGRAM is None:
        _PROGRAM = _build_program()
    return _PROGRAM


# ---------------------------------------------------------------- host side
def _pack_weights(W1, b1, Wz, bz, W2, b2, Ws):
    """Core-independent packed weights: packB minus pinv, plus per-core w2s."""
    packB = np.zeros((128, PW), np.float32)
    w1h = (
        W1.reshape(KX, 128, H).transpose(1, 0, 2).reshape(128, KX * H)
    ).astype(np.float16)
    packB[:, PK_W1[0] : PK_W1[1]] = w1h.view(np.float32)
    packB[:, PK_WZ[0] : PK_WZ[1]] = Wz.astype(np.float16).view(np.float32)
    packB[:H, PK_B1[0]] = b1
    packB[:, PK_BZ[0]] = bz
    return packB


def _prep_core_inputs(x16, z16, zpad16, packB_base, w2s_all, bs_all, idx_lists, core):
    """Per-core input map (grouped, padded, transposed, packed)."""
    xgT = np.zeros((IN, R), np.float16)
    zgT = np.empty((Z, R), np.float16)
    zgT[:] = zpad16[:, None]
    for s in range(G):
        k = core * G + s
        idx = idx_lists[k]
        n = len(idx)
        lo = s * B
        if n:
            xgT[:, lo : lo + n] = x16[idx].T
            zgT[:, lo : lo + n] = z16[idx].T
    packB = packB_base.copy()
    packB[:, PK_BS[0] : PK_BS[1]] = bs_all[core * G : (core + 1) * G].T
    w2s = w2s_all[core]
    return {"xgT": xgT, "zgT": zgT, "packB": packB, "w2s": w2s}


def _numpy_fallback(x, c, z, W1, b1, W2, b2, Wz, bz, Ws):
    x64 = x.astype(np.float64)
    fx = np.maximum(x64 @ W1.astype(np.float64) + b1, 0.0) @ W2.astype(
        np.float64
    ) + b2
    fz = z.astype(np.float64) @ Wz.astype(np.float64) + bz
    u = np.einsum("nd,nde->ne", fx, Ws.astype(np.float64)[c])

    def sp(v):
        return np.log1p(np.exp(-np.abs(v))) + np.maximum(v, 0.0)

    T = sp(np.einsum("ne,ne->n", u, fz))
    out = np.empty(N, np.float64)
    for k in range(C):
        idx = np.where(c == k)[0]
        if len(idx) == 0:
            continue
        Sk = sp(u[idx] @ fz[idx].T)
        neg = Sk.mean(axis=1)
        out[idx] = np.log(T[idx] + EPS) - np.log(neg + EPS)
    return out.astype(np.float32)


def _host_prepare(x, cf, z, W1, b1, W2, b2, Wz, bz, Ws, idx_lists):
    """Build per-core input maps; returns None if the fallback must run."""
    try:
        zpad = -np.linalg.solve(Wz.astype(np.float64).T, bz.astype(np.float64))
    except np.linalg.LinAlgError:
        return None
    if not np.all(np.isfinite(zpad)) or np.abs(zpad).max() > 1e3:
        return None
    zpad16 = zpad.astype(np.float16)

    packB_base = _pack_weights(W1, b1, Wz, bz, W2, b2, Ws)
    # fold the second MLP layer into each category's bilinear weight:
    # u = relu(h1) @ (W2 Ws[g]) + b2 Ws[g]
    Ws64 = Ws.astype(np.float64)
    w2s_full = np.einsum("he,cef->chf", W2.astype(np.float64), Ws64)
    bs_all = (b2.astype(np.float64) @ Ws64).astype(np.float32)  # [C, Z]
    w2s_all = [
        np.ascontiguousarray(
            w2s_full[core * G : (core + 1) * G]
            .transpose(1, 0, 2)
            .reshape(H, G * Z),
            dtype=np.float16,
        )
        for core in range(NCORES)
    ]
    x16 = x.astype(np.float16)
    z16 = z.astype(np.float16)

    return [
        _prep_core_inputs(
            x16, z16, zpad16, packB_base, w2s_all, bs_all, idx_lists, core
        )
        for core in range(NCORES)
    ]


def kernel(x, c, z, W1, b1, W2, b2, Wz, bz, Ws):
    x = np.ascontiguousarray(np.asarray(x), dtype=np.float32)
    z = np.ascontiguousarray(np.asarray(z), dtype=np.float32)
    W1 = np.ascontiguousarray(np.asarray(W1), dtype=np.float32)
    b1 = np.ascontiguousarray(np.asarray(b1), dtype=np.float32)
    W2 = np.ascontiguousarray(np.asarray(W2), dtype=np.float32)
    b2 = np.ascontiguousarray(np.asarray(b2), dtype=np.float32)
    Wz = np.ascontiguousarray(np.asarray(Wz), dtype=np.float32)
    bz = np.ascontiguousarray(np.asarray(bz), dtype=np.float32)
    Ws = np.ascontiguousarray(np.asarray(Ws), dtype=np.float32)
    cf = np.asarray(c).reshape(-1).astype(np.int64)

    idx_lists = [np.where(cf == k)[0] for k in range(C)]
    if max(len(i) for i in idx_lists) > JW:
        return _numpy_fallback(x, cf, z, W1, b1, W2, b2, Wz, bz, Ws)

    in_maps = _host_prepare(x, cf, z, W1, b1, W2, b2, Wz, bz, Ws, idx_lists)
    if in_maps is None:
        return _numpy_fallback(x, cf, z, W1, b1, W2, b2, Wz, bz, Ws)

    nc = get_program()
    res = run_bass_kernel_spmd(nc, in_maps, core_ids=list(range(NCORES)))

    # scalar epilogue in float64 on the host: y = log(softplus(d) + eps)
    #                                            - log(relu_sum / n + eps)
    out = np.empty(N, np.float32)
    for core in range(NCORES):
        dr = res.results[core]["dr"].astype(np.float64)  # [128, 2*NCHUNK]
        d = np.ascontiguousarray(dr[:, :NCHUNK].T).reshape(R)
        rel = np.ascontiguousarray(dr[:, NCHUNK:].T).reshape(R)
        T = np.log1p(np.exp(-np.abs(d))) + np.maximum(d, 0.0)
        for s in range(G):
            k = core * G + s
            idx = idx_lists[k]
            n = len(idx)
            if n:
                sl = slice(s * B, s * B + n)
                y = np.log(T[sl] + EPS) - np.log(rel[sl] / n + EPS)
                out[idx] = y.astype(np.float32)
    return out
